# revision 9
# baseline (speedup 1.0000x reference)
"""MoE dispatched linear (nn_DMoELinear) on 8 TRN2 NeuronCores.

out[t] = W[ids[t]] @ x[t] + b[ids[t]], computed in bf16 (matching the
reference, which casts x/W/b to bf16 before the grouped GEMM).

Strategy: expert parallelism. The host routes tokens by expert id
(the all-to-all dispatch, done host-side since kernel() receives full
inputs), core e runs expert e's GEMM for its tokens at shared static
capacity C = max_e count_e, and the host scatters rows back.

Per-core GEMM (hand-rolled Tile kernel, tokens on the moving/free dim
so no 128-padding of the token count is needed):
    yT[2048, C] = wT[2048, 2048].T @ xT[2048, C]  (+ bias, bf16 in,
    f32 PSUM accumulation, bf16 out)

Loop nest: token chunks (~C/3, <=512) outer, out-feature half (8 PSUM
banks) then out-feature block of 128; K=2048 contraction innermost as
16 SBUF-resident k-slabs. All of x and W are SBUF-resident (~98KB of
the 192KB per partition); DMA is ordered k-major for chunk 0 / m-half 0
first so the PE saturates during the load ramp.
"""

import numpy as np
import ml_dtypes

E = 8          # experts == cores
IN_F = 2048
OUT_F = 2048
P = 128
KO = IN_F // P    # 16 k-slabs
MO = OUT_F // P   # 16 out-feature blocks

_compile_cache = {}


def _chunks_of(C):
    n = -(-C // 512)          # ceil: minimum number of chunks of <=512
    base = C // n
    rem = C - base * n
    return [base + 1] * rem + [base] * (n - rem)


def _build_nc(C):
    """Build + compile the per-core Bass program for token capacity C.

    Two k-phases so the PE never waits on the DMA ramp: phase A
    accumulates k-slabs 0..7 into PSUM and evicts (+bias) to f32 SBUF via
    the Scalar engine; phase B accumulates slabs 8..15 and the Vector
    engine combines partials to bf16. Phase A only needs the first half
    of the 12.8MB input DMA but holds ~57us of PE work.
    """
    import concourse.mybir as mybir
    from concourse import bacc, tile

    chunks = _chunks_of(C)
    starts = np.concatenate([[0], np.cumsum(chunks)]).astype(int)
    NC = len(chunks)
    KH = KO // 2  # k-slabs per phase

    nc = bacc.Bacc("TRN2", target_bir_lowering=False, debug=False)
    xT = nc.dram_tensor("xT", [IN_F, C], mybir.dt.bfloat16, kind="ExternalInput")
    wT = nc.dram_tensor("wT", [IN_F, OUT_F], mybir.dt.bfloat16, kind="ExternalInput")
    bias = nc.dram_tensor("bias", [P, MO], mybir.dt.float32, kind="ExternalInput")
    yT = nc.dram_tensor("yT", [OUT_F, C], mybir.dt.bfloat16, kind="ExternalOutput")

    xv = xT.rearrange("(ko p) c -> p ko c", p=P)    # [128, 16, C]
    wv = wT.rearrange("(ko p) m -> p ko m", p=P)    # [128, 16, 2048]
    yv = yT.rearrange("(mo p) c -> p mo c", p=P)    # [128, 16, C]

    with tile.TileContext(nc) as tc:
        with (
            tc.tile_pool(name="weights", bufs=1) as wpool,
            tc.tile_pool(name="acts", bufs=1) as xpool,
            tc.tile_pool(name="acc", bufs=1) as apool,
            tc.tile_pool(name="out", bufs=6) as opool,
            tc.tile_pool(name="psum", bufs=8, space="PSUM") as ppool,
        ):
            bias_sb = wpool.tile([P, MO], mybir.dt.float32, tag="bias")
            nc.sync.dma_start(bias_sb[:], bias[:])

            # SBUF-resident inputs: whole-width x k-slabs (2*C-byte DMA
            # runs) and half-width w k-slabs (2KB runs). DMA engines
            # process each queue FIFO in issue order, so issue exactly
            # what the PE wavefront needs first: phase A consumes psums
            # (c, m) with m ascending, k-slabs 0..KH-1 — so x_k + w_k
            # lower half for k<KH go first, then the upper half, then
            # the same for the phase-B k-slabs.
            w_sb = [[None, None] for _ in range(KO)]
            x_sb = [None] * KO
            H = OUT_F // 2

            def load_x(k, eng=None):
                x_sb[k] = xpool.tile(
                    [P, C], mybir.dt.bfloat16, tag=f"x_{k}", name=f"x_{k}"
                )
                (eng or nc.sync).dma_start(x_sb[k][:], xv[:, k])

            def load_w(k, h, eng=None):
                w_sb[k][h] = wpool.tile(
                    [P, H], mybir.dt.bfloat16, tag=f"w_{k}_{h}", name=f"w_{k}_{h}"
                )
                (eng or nc.sync).dma_start(w_sb[k][h][:], wv[:, k, h * H : (h + 1) * H])

            # First two k-layers ride SWDGE (gpsimd): ~1us first-byte vs
            # the ~6us HWDGE ring-init latency, so the PE starts earlier.
            for k in range(2):
                load_x(k, nc.gpsimd)
                load_w(k, 0, nc.gpsimd)
            for k in range(2, KH):
                load_x(k)
                load_w(k, 0)
            for k in range(KH):
                load_w(k, 1)
            for k in range(KH, KO):
                load_x(k)
                load_w(k, 0)
            for k in range(KH, KO):
                load_w(k, 1)

            def w_slice(k, m):
                h, mi = divmod(m, MO // 2)
                return w_sb[k][h][:, mi * P : (mi + 1) * P]

            y_acc = [[None] * MO for _ in range(NC)]

            # Phase A: k-slabs 0..KH-1, partials (+bias) to f32 SBUF.
            for c, width in enumerate(chunks):
                for m in range(MO):
                    psum = ppool.tile([P, 512], mybir.dt.float32, tag="psum")
                    for k in range(KH):
                        nc.tensor.matmul(
                            psum[:, :width],
                            lhsT=w_slice(k, m),
                            rhs=x_sb[k][:, starts[c] : starts[c + 1]],
                            start=(k == 0),
                            stop=(k == KH - 1),
                        )
                    y_acc[c][m] = apool.tile(
                        [P, width], mybir.dt.float32,
                        tag=f"acc_{c}_{m}", name=f"acc_{c}_{m}",
                    )
                    nc.scalar.activation(
                        y_acc[c][m][:],
                        psum[:, :width],
                        mybir.ActivationFunctionType.Identity,
                        bias=bias_sb[:, m : m + 1],
                    )

            # Phase B: k-slabs KH..KO-1, combine with phase-A partials.
            for c, width in enumerate(chunks):
                for m in range(MO):
                    psum = ppool.tile([P, 512], mybir.dt.float32, tag="psum")
                    for k in range(KH, KO):
                        nc.tensor.matmul(
                            psum[:, :width],
                            lhsT=w_slice(k, m),
                            rhs=x_sb[k][:, starts[c] : starts[c + 1]],
                            start=(k == KH),
                            stop=(k == KO - 1),
                        )
                    y_sb = opool.tile([P, 512], mybir.dt.bfloat16, tag="y")
                    nc.vector.tensor_add(
                        y_sb[:, :width], psum[:, :width], y_acc[c][m][:]
                    )
                    nc.sync.dma_start(
                        yv[:, m, starts[c] : starts[c + 1]], y_sb[:, :width]
                    )
    nc.compile()
    return nc


def _route(x, ids):
    """Host-side dispatch: group token indices by expert."""
    ids_flat = np.asarray(ids).reshape(-1).astype(np.int64)
    order = np.argsort(ids_flat, kind="stable")
    counts = np.bincount(ids_flat, minlength=E)
    C = max(int(counts.max()), P)
    C = -(-C // 4) * 4  # round up to multiple of 4 for DMA alignment
    starts = np.zeros(E + 1, np.int64)
    np.cumsum(counts, out=starts[1:])
    return order, counts, starts, C


def _prepare(x, ids, weight, bias):
    x = np.asarray(x)
    weight = np.asarray(weight)
    bias = np.asarray(bias)
    out_shape = (*x.shape[:-1], weight.shape[1])
    x_flat = x.reshape(-1, x.shape[-1])
    order, counts, starts, C = _route(x, ids)

    bf16 = ml_dtypes.bfloat16
    w_bf = weight.astype(bf16)
    # match the reference: bias is cast to bf16 before the add
    b_f32 = bias.astype(bf16).astype(np.float32)

    in_maps = []
    for e in range(E):
        idx = order[starts[e] : starts[e + 1]]
        xT_e = np.zeros((IN_F, C), dtype=bf16)
        xT_e[:, : counts[e]] = np.ascontiguousarray(x_flat[idx].astype(bf16).T)
        wT_e = np.ascontiguousarray(w_bf[e].T)
        # bias[p, mo] = b[mo*128 + p]
        bias_e = np.ascontiguousarray(b_f32[e].reshape(MO, P).T)
        in_maps.append({"xT": xT_e, "wT": wT_e, "bias": bias_e})
    return in_maps, out_shape, x_flat.shape[0], order, counts, starts, C


def _gather(res, out_shape, T, order, counts, starts):
    bf16 = ml_dtypes.bfloat16
    out_flat = np.zeros((T, OUT_F), dtype=bf16)
    for e in range(E):
        idx = order[starts[e] : starts[e + 1]]
        yT_e = res.results[e]["yT"]  # [OUT_F, C]
        out_flat[idx] = yT_e[:, : counts[e]].T
    return out_flat.reshape(out_shape)


def kernel(x, ids, weight, bias):
    from concourse.bass_utils import run_bass_kernel_spmd

    in_maps, out_shape, T, order, counts, starts, C = _prepare(x, ids, weight, bias)
    if C not in _compile_cache:
        _compile_cache[C] = _build_nc(C)
    nc = _compile_cache[C]
    res = run_bass_kernel_spmd(nc, in_maps, core_ids=list(range(E)))
    return _gather(res, out_shape, T, order, counts, starts)


# Exposed for test.py: run with tracing and return (out, BassKernelResults).
def _run_traced(x, ids, weight, bias, tmpdir=None):
    from concourse.bass_utils import run_bass_kernel_spmd

    in_maps, out_shape, T, order, counts, starts, C = _prepare(x, ids, weight, bias)
    if C not in _compile_cache:
        _compile_cache[C] = _build_nc(C)
    nc = _compile_cache[C]
    res = run_bass_kernel_spmd(
        nc, in_maps, core_ids=list(range(E)), trace=True, tmpdir=tmpdir
    )
    return _gather(res, out_shape, T, order, counts, starts), res


# revision 11
# speedup vs baseline: 1.0148x; 1.0148x over previous
"""MoE dispatched linear (nn_DMoELinear) on 8 TRN2 NeuronCores.

out[t] = W[ids[t]] @ x[t] + b[ids[t]], computed in bf16 (matching the
reference, which casts x/W/b to bf16 before the grouped GEMM).

Strategy: expert parallelism. The host routes tokens by expert id
(the all-to-all dispatch, done host-side since kernel() receives full
inputs), core e runs expert e's GEMM for its tokens at shared static
capacity C = max_e count_e, and the host scatters rows back.

Per-core GEMM (hand-rolled Tile kernel, tokens on the moving/free dim
so no 128-padding of the token count is needed):
    yT[2048, C] = wT[2048, 2048].T @ xT[2048, C]  (+ bias, bf16 in,
    f32 PSUM accumulation, bf16 out)

Loop nest: token chunks (~C/3, <=512) outer, out-feature half (8 PSUM
banks) then out-feature block of 128; K=2048 contraction innermost as
16 SBUF-resident k-slabs. All of x and W are SBUF-resident (~98KB of
the 192KB per partition); DMA is ordered k-major for chunk 0 / m-half 0
first so the PE saturates during the load ramp.
"""

import numpy as np
import ml_dtypes

E = 8          # experts == cores
IN_F = 2048
OUT_F = 2048
P = 128
KO = IN_F // P    # 16 k-slabs
MO = OUT_F // P   # 16 out-feature blocks

_compile_cache = {}


def _chunks_of(C, max_w=512):
    n = -(-C // max_w)        # ceil: minimum number of chunks of <=max_w
    base = C // n
    rem = C - base * n
    return [base + 1] * rem + [base] * (n - rem)


def _build_nc(C):
    """Build + compile the per-core Bass program for token capacity C.

    Two k-phases so the PE never waits on the DMA ramp: phase A
    accumulates k-slabs 0..7 into PSUM and evicts (+bias) to f32 SBUF via
    the Scalar engine; phase B accumulates slabs 8..15 and the Vector
    engine combines partials to bf16. Phase A only needs the first half
    of the 12.8MB input DMA but holds ~57us of PE work.
    """
    import concourse.mybir as mybir
    from concourse import bacc, tile

    chunks = _chunks_of(C)
    starts = np.concatenate([[0], np.cumsum(chunks)]).astype(int)
    NC = len(chunks)
    KH = KO // 2  # k-slabs per phase

    nc = bacc.Bacc("TRN2", target_bir_lowering=False, debug=False)
    xT = nc.dram_tensor("xT", [IN_F, C], mybir.dt.bfloat16, kind="ExternalInput")
    wT = nc.dram_tensor("wT", [IN_F, OUT_F], mybir.dt.bfloat16, kind="ExternalInput")
    bias = nc.dram_tensor("bias", [P, MO], mybir.dt.float32, kind="ExternalInput")
    yT = nc.dram_tensor("yT", [OUT_F, C], mybir.dt.bfloat16, kind="ExternalOutput")

    xv = xT.rearrange("(ko p) c -> p ko c", p=P)    # [128, 16, C]
    wv = wT.rearrange("(ko p) m -> p ko m", p=P)    # [128, 16, 2048]
    yv = yT.rearrange("(mo p) c -> p mo c", p=P)    # [128, 16, C]

    with tile.TileContext(nc) as tc:
        with (
            tc.tile_pool(name="weights", bufs=1) as wpool,
            tc.tile_pool(name="acts", bufs=1) as xpool,
            tc.tile_pool(name="acc", bufs=1) as apool,
            tc.tile_pool(name="out", bufs=6) as opool,
            tc.tile_pool(name="psum", bufs=8, space="PSUM") as ppool,
        ):
            bias_sb = wpool.tile([P, MO], mybir.dt.float32, tag="bias")
            nc.sync.dma_start(bias_sb[:], bias[:])

            # SBUF-resident inputs: whole-width x k-slabs (2*C-byte DMA
            # runs) and half-width w k-slabs (2KB runs). DMA engines
            # process each queue FIFO in issue order, so issue exactly
            # what the PE wavefront needs first: phase A consumes psums
            # (c, m) with m ascending, k-slabs 0..KH-1 — so x_k + w_k
            # lower half for k<KH go first, then the upper half, then
            # the same for the phase-B k-slabs.
            w_sb = [[None, None] for _ in range(KO)]
            x_sb = [None] * KO
            H = OUT_F // 2

            def load_x(k, eng=None):
                x_sb[k] = xpool.tile(
                    [P, C], mybir.dt.bfloat16, tag=f"x_{k}", name=f"x_{k}"
                )
                (eng or nc.sync).dma_start(x_sb[k][:], xv[:, k])

            def load_w(k, h, eng=None):
                w_sb[k][h] = wpool.tile(
                    [P, H], mybir.dt.bfloat16, tag=f"w_{k}_{h}", name=f"w_{k}_{h}"
                )
                (eng or nc.sync).dma_start(w_sb[k][h][:], wv[:, k, h * H : (h + 1) * H])

            for k in range(KH):
                load_x(k)
                load_w(k, 0)
            for k in range(KH):
                load_w(k, 1)
            for k in range(KH, KO):
                load_x(k)
                load_w(k, 0)
            for k in range(KH, KO):
                load_w(k, 1)

            def w_slice(k, m):
                h, mi = divmod(m, MO // 2)
                return w_sb[k][h][:, mi * P : (mi + 1) * P]

            y_acc = [[None] * MO for _ in range(NC)]

            # Phase A: k-slabs 0..KH-1, partials (+bias) to f32 SBUF.
            for c, width in enumerate(chunks):
                for m in range(MO):
                    psum = ppool.tile([P, 512], mybir.dt.float32, tag="psum")
                    for k in range(KH):
                        nc.tensor.matmul(
                            psum[:, :width],
                            lhsT=w_slice(k, m),
                            rhs=x_sb[k][:, starts[c] : starts[c + 1]],
                            start=(k == 0),
                            stop=(k == KH - 1),
                        )
                    y_acc[c][m] = apool.tile(
                        [P, width], mybir.dt.float32,
                        tag=f"acc_{c}_{m}", name=f"acc_{c}_{m}",
                    )
                    nc.scalar.activation(
                        y_acc[c][m][:],
                        psum[:, :width],
                        mybir.ActivationFunctionType.Identity,
                        bias=bias_sb[:, m : m + 1],
                    )

            # Phase B: k-slabs KH..KO-1, combine with phase-A partials.
            for c, width in enumerate(chunks):
                for m in range(MO):
                    psum = ppool.tile([P, 512], mybir.dt.float32, tag="psum")
                    for k in range(KH, KO):
                        nc.tensor.matmul(
                            psum[:, :width],
                            lhsT=w_slice(k, m),
                            rhs=x_sb[k][:, starts[c] : starts[c + 1]],
                            start=(k == KH),
                            stop=(k == KO - 1),
                        )
                    y_sb = opool.tile([P, 512], mybir.dt.bfloat16, tag="y")
                    nc.vector.tensor_add(
                        y_sb[:, :width], psum[:, :width], y_acc[c][m][:]
                    )
                    nc.sync.dma_start(
                        yv[:, m, starts[c] : starts[c + 1]], y_sb[:, :width]
                    )
    nc.compile()
    return nc


def _route(x, ids):
    """Host-side dispatch: group token indices by expert."""
    ids_flat = np.asarray(ids).reshape(-1).astype(np.int64)
    order = np.argsort(ids_flat, kind="stable")
    counts = np.bincount(ids_flat, minlength=E)
    C = max(int(counts.max()), P)
    C = -(-C // 4) * 4  # round up to multiple of 4 for DMA alignment
    starts = np.zeros(E + 1, np.int64)
    np.cumsum(counts, out=starts[1:])
    return order, counts, starts, C


def _prepare(x, ids, weight, bias):
    x = np.asarray(x)
    weight = np.asarray(weight)
    bias = np.asarray(bias)
    out_shape = (*x.shape[:-1], weight.shape[1])
    x_flat = x.reshape(-1, x.shape[-1])
    order, counts, starts, C = _route(x, ids)

    bf16 = ml_dtypes.bfloat16
    w_bf = weight.astype(bf16)
    # match the reference: bias is cast to bf16 before the add
    b_f32 = bias.astype(bf16).astype(np.float32)

    in_maps = []
    for e in range(E):
        idx = order[starts[e] : starts[e + 1]]
        xT_e = np.zeros((IN_F, C), dtype=bf16)
        xT_e[:, : counts[e]] = np.ascontiguousarray(x_flat[idx].astype(bf16).T)
        wT_e = np.ascontiguousarray(w_bf[e].T)
        # bias[p, mo] = b[mo*128 + p]
        bias_e = np.ascontiguousarray(b_f32[e].reshape(MO, P).T)
        in_maps.append({"xT": xT_e, "wT": wT_e, "bias": bias_e})
    return in_maps, out_shape, x_flat.shape[0], order, counts, starts, C


def _gather(res, out_shape, T, order, counts, starts):
    bf16 = ml_dtypes.bfloat16
    out_flat = np.zeros((T, OUT_F), dtype=bf16)
    for e in range(E):
        idx = order[starts[e] : starts[e + 1]]
        yT_e = res.results[e]["yT"]  # [OUT_F, C]
        out_flat[idx] = yT_e[:, : counts[e]].T
    return out_flat.reshape(out_shape)


def kernel(x, ids, weight, bias):
    from concourse.bass_utils import run_bass_kernel_spmd

    in_maps, out_shape, T, order, counts, starts, C = _prepare(x, ids, weight, bias)
    if C not in _compile_cache:
        _compile_cache[C] = _build_nc(C)
    nc = _compile_cache[C]
    res = run_bass_kernel_spmd(nc, in_maps, core_ids=list(range(E)))
    return _gather(res, out_shape, T, order, counts, starts)


# Exposed for test.py: run with tracing and return (out, BassKernelResults).
def _run_traced(x, ids, weight, bias, tmpdir=None):
    from concourse.bass_utils import run_bass_kernel_spmd

    in_maps, out_shape, T, order, counts, starts, C = _prepare(x, ids, weight, bias)
    if C not in _compile_cache:
        _compile_cache[C] = _build_nc(C)
    nc = _compile_cache[C]
    res = run_bass_kernel_spmd(
        nc, in_maps, core_ids=list(range(E)), trace=True, tmpdir=tmpdir
    )
    return _gather(res, out_shape, T, order, counts, starts), res
